# revision 7
# baseline (speedup 1.0000x reference)
"""Bidirectional batched GRU over ragged graph sequences — TRN2 Bass/Tile kernel.

Strategy (8 NeuronCores, data-parallel over B=256 graphs -> 32 graphs/core):
  * Both directions' recurrent chains run merged on each core, stacked along
    PSUM/SBUF partitions so the PE's col-group tiling (tile_position) packs
    4 concurrent M=32 matmuls per wave and the elementwise ops process both
    directions in single [64, 512] / [128, 512] instructions.
  * Per step t, PSUM bank `rz` [128,512] holds the 4 groups (f-r, b-r, f-z,
    b-z) = x@wi + h@wh + (bi+bh) accumulated by the PE; bank `nn` holds
    (f-ghn, b-ghn, f-sxn, b-sxn) with ghn = h@wh_n + bh_n, sxn = x@wi_n + bi_n.
  * Gates: sigmoid reads PSUM directly; u = 1-z computed as sigmoid(-s_z).
    n = tanh(sxn + r*ghn);  h' = u*n + z*h.
  * h' is transposed back to [h, batch] layout via 8 tiny PE transposes for
    the next step's stationary operand.
  * Ragged mean-pool: per-step masked accumulation into a persistent PSUM
    bank via diag(mask_t) matmuls; final scale by 1/node_size.
  * h0 = max over padded time = strided tensor_reduce over x.T in SBUF.

All SBUF data is bf16 (PE runs bf16 at 1 cycle/row); PSUM accumulates fp32.
"""
import numpy as np
import ml_dtypes as md
from contextlib import ExitStack

import concourse.bass as bass
import concourse.tile as tile
from concourse import bacc, mybir
from concourse.bass_utils import run_bass_kernel_spmd
from concourse.tile_rust import add_dep_helper

F32 = mybir.dt.float32
BF16 = mybir.dt.bfloat16
AF = mybir.ActivationFunctionType
ALU = mybir.AluOpType

B, H, T = 256, 512, 64
NC = 8
BG = B // NC          # 32 graphs per core
H3 = 3 * H
KC = H // 128         # 4 contraction chunks
NT = H3 // 512        # 3 gate chunks (r, z, n)

_PROGRAM_CACHE = {}


def dep(mm, on):
    add_dep_helper(mm.ins, on.ins, sync=False, reason="psum group order")
    return mm


def build_program(T_steps=T, use_gpsimd=True, repeat=1):
    nc = bacc.Bacc("TRN2", target_bir_lowering=False, debug=False)

    xT = nc.dram_tensor("xT", [H, BG * T], BF16, kind="ExternalInput").ap()
    w_all = nc.dram_tensor("w_all", [4, H, H3], BF16, kind="ExternalInput").ap()
    bias_rz = nc.dram_tensor("bias_rz", [4, 512], BF16, kind="ExternalInput").ap()
    bias_n = nc.dram_tensor("bias_n", [4, 512], BF16, kind="ExternalInput").ap()
    ind4_d = nc.dram_tensor("ind4", [4, 128], BF16, kind="ExternalInput").ap()
    maskdiag = nc.dram_tensor("maskdiag", [2 * BG, T, BG], BF16, kind="ExternalInput").ap()
    inv_ns = nc.dram_tensor("inv_ns", [2 * BG, 1], F32, kind="ExternalInput").ap()
    ident_d = nc.dram_tensor("ident", [128, 128], BF16, kind="ExternalInput").ap()
    out_d = nc.dram_tensor("out", [2 * BG, H], F32, kind="ExternalOutput").ap()

    with tile.TileContext(nc) as tc, ExitStack() as ctx:
        cst = ctx.enter_context(tc.tile_pool(name="cst", bufs=1))
        hTp = ctx.enter_context(tc.tile_pool(name="hTp", bufs=3))
        hp = ctx.enter_context(tc.tile_pool(name="hp", bufs=3))
        ew = ctx.enter_context(tc.tile_pool(name="ew", bufs=2))
        rzp = ctx.enter_context(tc.tile_pool(name="rzp", bufs=2, space="PSUM"))
        nnp = ctx.enter_context(tc.tile_pool(name="nnp", bufs=2, space="PSUM"))
        htp = ctx.enter_context(tc.tile_pool(name="htp", bufs=2, space="PSUM"))
        accp = ctx.enter_context(tc.tile_pool(name="accp", bufs=1, space="PSUM"))

        # ---- constants / inputs to SBUF
        xs = cst.tile([128, KC, BG * T], BF16)           # x.T  [h%128, h//128, (b,t)]
        nc.sync.dma_start(xs[:], xT.rearrange("(c p) r -> p c r", p=128))
        ws = cst.tile([128, 4, KC, H3], BF16)            # wiT_f, whT_f, wiT_b, whT_b
        nc.sync.dma_start(ws[:], w_all.rearrange("m (c p) n -> p m c n", p=128))
        brz = cst.tile([4, 512], BF16)
        nc.sync.dma_start(brz[:], bias_rz[:])
        bnn = cst.tile([4, 512], BF16)
        nc.sync.dma_start(bnn[:], bias_n[:])
        ind4 = cst.tile([4, 128], BF16)
        nc.sync.dma_start(ind4[:], ind4_d[:])
        mdg = cst.tile([2 * BG, T, BG], BF16)
        nc.sync.dma_start(mdg[:], maskdiag[:])
        invn = cst.tile([2 * BG, 1], F32)
        nc.sync.dma_start(invn[:], inv_ns[:])
        ident = cst.tile([128, 128], BF16)
        nc.sync.dma_start(ident[:], ident_d[:])

        def x_lhsT(d, c, t):
            # stationary operand [128, BG] for direction d, k-chunk c, step t
            tt = t if d == 0 else T - 1 - t
            return xs[:, c, :].rearrange("p (b t) -> p t b", t=T)[:, tt, :]

        def w_rhs(m, c, j):
            return ws[:, m, c, 512 * j : 512 * (j + 1)]

        for _rep in range(repeat):
            body(nc, tc, ctx, T_steps, use_gpsimd, xs, ws, brz, bnn, ind4, mdg,
                 invn, ident, out_d, x_lhsT, w_rhs, cst, hTp, hp, ew, rzp, nnp,
                 htp, accp)

    nc.compile()
    return nc


def body(nc, tc, ctx, T_steps, use_gpsimd, xs, ws, brz, bnn, ind4, mdg, invn,
         ident, out_d, x_lhsT, w_rhs, cst, hTp, hp, ew, rzp, nnp, htp, accp):
    if True:
        # ---- h0 = max over padded time, directly in transposed layout
        hT_f = hTp.tile([128, 128], BF16, tag="hTf")
        for c in range(KC):
            nc.vector.tensor_reduce(
                hT_f[:, 32 * c : 32 * (c + 1)],
                xs[:, c, :].rearrange("p (b t) -> p b t", t=T),
                mybir.AxisListType.X, ALU.max,
            )
        hT_b = hTp.tile([128, 128], BF16, tag="hTb")
        nc.vector.tensor_copy(hT_b[:], hT_f[:])

        # h0 in [batch, h] layout via PE transposes
        h0ps = accp.tile([BG, H], BF16, tag="h0ps")
        for c in range(KC):
            nc.tensor.transpose(
                h0ps[:, 128 * c : 128 * (c + 1)],
                hT_f[:, 32 * c : 32 * (c + 1)],
                ident[:],
            )
        h_cur = hp.tile([2 * BG, H], BF16, tag="h")
        nc.vector.tensor_copy(h_cur[0:BG, :], h0ps[:])
        nc.vector.tensor_copy(h_cur[BG : 2 * BG, :], h0ps[:])

        acc = accp.tile([2 * BG, 512], F32)

        pend = None  # (h_new, t) awaiting transposes/acc emission
        for t in range(T_steps):
            rz = rzp.tile([128, 512], F32, tag="rz")
            nn = nnp.tile([128, 512], F32, tag="nn")

            # bias MMs open the accumulation groups (write every element)
            rz_b = nc.tensor.matmul(rz[:], ind4[:], brz[:], start=True, stop=False,
                                    skip_group_check=True)
            nn_b = nc.tensor.matmul(nn[:], ind4[:], bnn[:], start=True, stop=False,
                                    skip_group_check=True)

            # x-projection MMs (independent of the recurrent chain)
            # rz groups: 0=f-r, 1=b-r, 2=f-z, 3=b-z ; nn groups 2,3 = f-sxn, b-sxn
            for c in range(KC):
                for g, (d, j) in enumerate([(0, 0), (1, 0), (0, 1), (1, 1)]):
                    m = 2 * d  # wi of direction d
                    dep(nc.tensor.matmul(
                        rz[32 * g : 32 * (g + 1), :], x_lhsT(d, c, t), w_rhs(m, c, j),
                        start=False, stop=False, skip_group_check=True,
                        tile_position=(0, 32 * g)), rz_b)
                for g, d in ((2, 0), (3, 1)):
                    dep(nc.tensor.matmul(
                        nn[32 * g : 32 * (g + 1), :], x_lhsT(d, c, t), w_rhs(2 * d, c, 2),
                        start=False, stop=(c == KC - 1), skip_group_check=True,
                        tile_position=(0, 32 * g)), nn_b)

            # deferred transposes + masked-mean accumulate for step t-1
            if pend is not None:
                hprev, tprev = pend
                # HW quirk: transpose-mode matmuls crash with base-partition-32
                # inputs, so stage the b-direction rows down to partition 0 first.
                hb0 = ew.tile([BG, H], BF16, tag="hb0")
                nc.vector.tensor_copy(hb0[:], hprev[BG : 2 * BG, :])
                htps = htp.tile([128, 2 * 128], BF16, tag="ht")
                for d in range(2):
                    src = hprev if d == 0 else hb0
                    for c in range(KC):
                        nc.tensor.transpose(
                            htps[:, 128 * d + 32 * c : 128 * d + 32 * (c + 1)],
                            src[0:BG, 128 * c : 128 * (c + 1)],
                            ident[0:BG, 0:BG],
                        )
                mm_a = nc.tensor.matmul(acc[0:BG, :], mdg[0:BG, tprev, :], hprev[0:BG, :],
                                        start=(tprev == 0), stop=False,
                                        skip_group_check=True, tile_position=(0, 0))
                mm_b = nc.tensor.matmul(acc[BG : 2 * BG, :], mdg[BG : 2 * BG, tprev, :],
                                        hprev[BG : 2 * BG, :],
                                        start=(tprev == 0), stop=False,
                                        skip_group_check=True, tile_position=(32, 32))
                if tprev == 0:
                    acc_start = (mm_a, mm_b)
                else:
                    dep(mm_a, acc_start[0])
                    dep(mm_b, acc_start[1])
                hT_f = hTp.tile([128, 128], BF16, tag="hTf")
                nc.vector.tensor_copy(hT_f[:], htps[:, 0:128])
                hT_b = hTp.tile([128, 128], BF16, tag="hTb")
                nc.vector.tensor_copy(hT_b[:], htps[:, 128:256])

            # recurrent MMs
            for c in range(KC):
                for g, (hT, j) in enumerate([(hT_f, 0), (hT_b, 0), (hT_f, 1), (hT_b, 1)]):
                    m = 1 + 2 * (g % 2)  # wh of direction
                    dep(nc.tensor.matmul(
                        rz[32 * g : 32 * (g + 1), :],
                        hT[:, 32 * c : 32 * (c + 1)], w_rhs(m, c, j),
                        start=False, stop=(c == KC - 1), skip_group_check=True,
                        tile_position=(0, 32 * g)), rz_b)
                for g, hT in ((0, hT_f), (1, hT_b)):
                    dep(nc.tensor.matmul(
                        nn[32 * g : 32 * (g + 1), :],
                        hT[:, 32 * c : 32 * (c + 1)], w_rhs(1 + 2 * g, c, 2),
                        start=False, stop=(c == KC - 1), skip_group_check=True,
                        tile_position=(0, 32 * g)), nn_b)

            # ---- elementwise
            rzs = ew.tile([128, 512], BF16, tag="rzs")
            nc.scalar.activation(rzs[:], rz[:], AF.Sigmoid)
            u0 = ew.tile([2 * BG, 512], BF16, tag="u0")
            nc.scalar.activation(u0[:], rz[64:128, :], AF.Sigmoid, scale=-1.0)
            z0 = ew.tile([2 * BG, 512], BF16, tag="z0")
            nc.vector.tensor_copy(z0[:], rzs[64:128, :])
            b2 = ew.tile([2 * BG, 512], BF16, tag="b2")
            nc.vector.tensor_tensor(b2[:], rzs[0:64, :], nn[0:64, :], ALU.mult)
            cc = ew.tile([2 * BG, 512], BF16, tag="cc")
            nc.vector.tensor_tensor(cc[:], b2[:], nn[64:128, :], ALU.add)
            n_t = ew.tile([2 * BG, 512], BF16, tag="nt")
            nc.scalar.activation(n_t[:], cc[:], AF.Tanh)
            w_t = ew.tile([2 * BG, 512], BF16, tag="wt")
            eng_w = nc.gpsimd if use_gpsimd else nc.vector
            eng_w.tensor_tensor(w_t[:], z0[:], h_cur[:], ALU.mult)
            v_t = ew.tile([2 * BG, 512], BF16, tag="vt")
            nc.vector.tensor_tensor(v_t[:], u0[:], n_t[:], ALU.mult)
            h_new = hp.tile([2 * BG, H], BF16, tag="h")
            nc.vector.tensor_tensor(h_new[:], v_t[:], w_t[:], ALU.add)

            pend = (h_new, t)
            h_cur = h_new

        # final masked accumulate for t=63
        hprev, tprev = pend
        dep(nc.tensor.matmul(acc[0:BG, :], mdg[0:BG, tprev, :], hprev[0:BG, :],
                             start=False, stop=True, skip_group_check=True,
                             tile_position=(0, 0)), acc_start[0])
        dep(nc.tensor.matmul(acc[BG : 2 * BG, :], mdg[BG : 2 * BG, tprev, :],
                             hprev[BG : 2 * BG, :], start=False, stop=True,
                             skip_group_check=True, tile_position=(32, 32)),
            acc_start[1])

        outs = cst.tile([2 * BG, H], F32)
        nc.vector.tensor_scalar_mul(outs[:], acc[:], invn[:])
        nc.sync.dma_start(out_d[:], outs[:])


def _prep_inputs(atom_message, w_ih, w_hh, b_ih, b_hh, node_size):
    """Host-side sharding/layout prep. Returns per-core input maps."""
    atom_message = np.asarray(atom_message, dtype=np.float32)
    w_ih = np.asarray(w_ih, dtype=np.float32)
    w_hh = np.asarray(w_hh, dtype=np.float32)
    b_ih = np.asarray(b_ih, dtype=np.float32)
    b_hh = np.asarray(b_hh, dtype=np.float32)
    ns = np.asarray(node_size, dtype=np.int64)

    # ragged -> padded [B, T, H]
    padded = np.zeros((B, T, H), dtype=np.float32)
    starts = np.concatenate([[0], np.cumsum(ns)[:-1]])
    for b in range(B):
        padded[b, : ns[b]] = atom_message[starts[b] : starts[b] + ns[b]]

    # weights (shared across cores)
    w_all = np.stack([w_ih[0].T, w_hh[0].T, w_ih[1].T, w_hh[1].T]).astype(md.bfloat16)
    bias_rz = np.stack([
        b_ih[0, 0:512] + b_hh[0, 0:512],       # f-r
        b_ih[1, 0:512] + b_hh[1, 0:512],       # b-r
        b_ih[0, 512:1024] + b_hh[0, 512:1024], # f-z
        b_ih[1, 512:1024] + b_hh[1, 512:1024], # b-z
    ]).astype(md.bfloat16)
    bias_n = np.stack([
        b_hh[0, 1024:1536], b_hh[1, 1024:1536],  # ghn f, b
        b_ih[0, 1024:1536], b_ih[1, 1024:1536],  # sxn f, b
    ]).astype(md.bfloat16)
    ind4 = np.zeros((4, 128), dtype=np.float32)
    for g in range(4):
        ind4[g, 32 * g : 32 * (g + 1)] = 1.0
    ind4 = ind4.astype(md.bfloat16)
    ident = np.eye(128, dtype=md.bfloat16)

    tgrid = np.arange(T)
    in_maps = []
    for i in range(NC):
        sl = slice(BG * i, BG * (i + 1))
        px = padded[sl]                                   # [32, 64, 512]
        xT = np.ascontiguousarray(px.transpose(2, 0, 1).reshape(H, BG * T))
        nsc = ns[sl]
        mask_f = (tgrid[None, :] < nsc[:, None]).astype(np.float32)       # [32, 64]
        mask_b = (tgrid[None, :] >= (T - nsc)[:, None]).astype(np.float32)
        mdg = np.zeros((2 * BG, T, BG), dtype=np.float32)
        for b in range(BG):
            mdg[b, :, b] = mask_f[b]
            mdg[BG + b, :, b] = mask_b[b]
        inv = np.concatenate([1.0 / nsc, 1.0 / nsc]).astype(np.float32)[:, None]
        in_maps.append({
            "xT": xT.astype(md.bfloat16),
            "w_all": w_all,
            "bias_rz": bias_rz,
            "bias_n": bias_n,
            "ind4": ind4,
            "maskdiag": mdg.astype(md.bfloat16),
            "inv_ns": inv,
            "ident": ident,
        })
    return in_maps


def kernel(atom_message, w_ih, w_hh, b_ih, b_hh, node_size, _sim=False, _trace=False):
    in_maps = _prep_inputs(atom_message, w_ih, w_hh, b_ih, b_hh, node_size)
    import os
    key = (int(os.environ.get("T_STEPS", T)), int(os.environ.get("USE_GPSIMD", 1)), int(os.environ.get("REPEAT", 1)))
    if key not in _PROGRAM_CACHE:
        _PROGRAM_CACHE[key] = build_program(T_steps=key[0], use_gpsimd=bool(key[1]), repeat=key[2])
    nc = _PROGRAM_CACHE[key]

    if _sim:
        from concourse.bass_interp import CoreSim
        sim = CoreSim(nc, require_finite=False, require_nnan=False)
        for k, v in in_maps[0].items():
            sim.tensor(k)[:] = v
        sim.simulate(check_with_hw=False)
        results = [{"out": np.array(sim.tensor("out"))}]
        # simulate only core 0; replicate shape for gather below
        results = results + [None] * (NC - 1)
        res_obj = None
    else:
        res_obj = run_bass_kernel_spmd(nc, in_maps, list(range(NC)), trace=_trace)
        results = res_obj.results

    out = np.zeros((B, 2 * H), dtype=np.float32)
    for i in range(NC):
        r = results[i]
        if r is None:
            continue
        sl = slice(BG * i, BG * (i + 1))
        out[sl, 0:H] = r["out"][0:BG]
        out[sl, H : 2 * H] = r["out"][BG : 2 * BG]
    if _sim:
        return out, None
    return out if not _trace else (out, res_obj)


# revision 8
# speedup vs baseline: 18.9046x; 18.9046x over previous
"""Bidirectional batched GRU over ragged graph sequences — TRN2 Bass/Tile kernel.

Strategy (8 NeuronCores, data-parallel over B=256 graphs -> 32 graphs/core):
  * Both directions' recurrent chains run merged on each core, stacked along
    PSUM/SBUF partitions so the PE's col-group tiling (tile_position) packs
    4 concurrent M=32 matmuls per wave and the elementwise ops process both
    directions in single [64, 512] / [128, 512] instructions.
  * Per step t, PSUM bank `rz` [128,512] holds the 4 groups (f-r, b-r, f-z,
    b-z) = x@wi + h@wh + (bi+bh) accumulated by the PE; bank `nn` holds
    (f-ghn, b-ghn, f-sxn, b-sxn) with ghn = h@wh_n + bh_n, sxn = x@wi_n + bi_n.
  * Gates: sigmoid reads PSUM directly; u = 1-z computed as sigmoid(-s_z).
    n = tanh(sxn + r*ghn);  h' = u*n + z*h.
  * h' is transposed back to [h, batch] layout via 8 tiny PE transposes for
    the next step's stationary operand.
  * Ragged mean-pool: per-step masked accumulation into a persistent PSUM
    bank via diag(mask_t) matmuls; final scale by 1/node_size.
  * h0 = max over padded time = strided tensor_reduce over x.T in SBUF.

All SBUF data is bf16 (PE runs bf16 at 1 cycle/row); PSUM accumulates fp32.
"""
import numpy as np
import ml_dtypes as md
from contextlib import ExitStack

import concourse.bass as bass
import concourse.tile as tile
from concourse import bacc, mybir
from concourse.bass_utils import run_bass_kernel_spmd
from concourse.tile_rust import add_dep_helper

F32 = mybir.dt.float32
BF16 = mybir.dt.bfloat16
AF = mybir.ActivationFunctionType
ALU = mybir.AluOpType

B, H, T = 256, 512, 64
NC = 8
BG = B // NC          # 32 graphs per core
H3 = 3 * H
KC = H // 128         # 4 contraction chunks
NT = H3 // 512        # 3 gate chunks (r, z, n)

_PROGRAM_CACHE = {}


def dep(mm, on):
    add_dep_helper(mm.ins, on.ins, sync=False, reason="psum group order")
    return mm


def build_program(T_steps=T, use_gpsimd=True, repeat=1, loop_n=None):
    nc = bacc.Bacc("TRN2", target_bir_lowering=False, debug=False)

    xT = nc.dram_tensor("xT", [H, BG * T], BF16, kind="ExternalInput").ap()
    w_all = nc.dram_tensor("w_all", [4, H, H3], BF16, kind="ExternalInput").ap()
    bias_rz = nc.dram_tensor("bias_rz", [4, 512], BF16, kind="ExternalInput").ap()
    bias_n = nc.dram_tensor("bias_n", [4, 512], BF16, kind="ExternalInput").ap()
    ind4_d = nc.dram_tensor("ind4", [4, 128], BF16, kind="ExternalInput").ap()
    maskdiag = nc.dram_tensor("maskdiag", [2 * BG, T, BG], BF16, kind="ExternalInput").ap()
    inv_ns = nc.dram_tensor("inv_ns", [2 * BG, 1], F32, kind="ExternalInput").ap()
    ident_d = nc.dram_tensor("ident", [128, 128], BF16, kind="ExternalInput").ap()
    out_d = nc.dram_tensor("out", [2 * BG, H], F32, kind="ExternalOutput").ap()

    with tile.TileContext(nc) as tc, ExitStack() as ctx:
        cst = ctx.enter_context(tc.tile_pool(name="cst", bufs=1))
        hTp = ctx.enter_context(tc.tile_pool(name="hTp", bufs=3))
        hp = ctx.enter_context(tc.tile_pool(name="hp", bufs=3))
        ew = ctx.enter_context(tc.tile_pool(name="ew", bufs=2))
        rzp = ctx.enter_context(tc.tile_pool(name="rzp", bufs=2, space="PSUM"))
        nnp = ctx.enter_context(tc.tile_pool(name="nnp", bufs=2, space="PSUM"))
        htp = ctx.enter_context(tc.tile_pool(name="htp", bufs=2, space="PSUM"))
        accp = ctx.enter_context(tc.tile_pool(name="accp", bufs=1, space="PSUM"))

        # ---- constants / inputs to SBUF
        xs = cst.tile([128, KC, BG * T], BF16)           # x.T  [h%128, h//128, (b,t)]
        nc.sync.dma_start(xs[:], xT.rearrange("(c p) r -> p c r", p=128))
        ws = cst.tile([128, 4, KC, H3], BF16)            # wiT_f, whT_f, wiT_b, whT_b
        nc.sync.dma_start(ws[:], w_all.rearrange("m (c p) n -> p m c n", p=128))
        brz = cst.tile([4, 512], BF16)
        nc.sync.dma_start(brz[:], bias_rz[:])
        bnn = cst.tile([4, 512], BF16)
        nc.sync.dma_start(bnn[:], bias_n[:])
        ind4 = cst.tile([4, 128], BF16)
        nc.sync.dma_start(ind4[:], ind4_d[:])
        mdg = cst.tile([2 * BG, T, BG], BF16)
        nc.sync.dma_start(mdg[:], maskdiag[:])
        invn = cst.tile([2 * BG, 1], F32)
        nc.sync.dma_start(invn[:], inv_ns[:])
        ident = cst.tile([128, 128], BF16)
        nc.sync.dma_start(ident[:], ident_d[:])

        def x_lhsT(d, c, t):
            # stationary operand [128, BG] for direction d, k-chunk c, step t
            tt = t if d == 0 else T - 1 - t
            return xs[:, c, :].rearrange("p (b t) -> p t b", t=T)[:, tt, :]

        def w_rhs(m, c, j):
            return ws[:, m, c, 512 * j : 512 * (j + 1)]

        if loop_n is not None:
            with tc.For_i(0, loop_n, 1):
                body(nc, tc, ctx, T_steps, use_gpsimd, xs, ws, brz, bnn, ind4,
                     mdg, invn, ident, out_d, x_lhsT, w_rhs, cst, hTp, hp, ew,
                     rzp, nnp, htp, accp)
        else:
            for _rep in range(repeat):
                body(nc, tc, ctx, T_steps, use_gpsimd, xs, ws, brz, bnn, ind4,
                     mdg, invn, ident, out_d, x_lhsT, w_rhs, cst, hTp, hp, ew,
                     rzp, nnp, htp, accp)

    nc.compile()
    return nc


def body(nc, tc, ctx, T_steps, use_gpsimd, xs, ws, brz, bnn, ind4, mdg, invn,
         ident, out_d, x_lhsT, w_rhs, cst, hTp, hp, ew, rzp, nnp, htp, accp):
    if True:
        # ---- h0 = max over padded time, directly in transposed layout
        hT_f = hTp.tile([128, 128], BF16, tag="hTf")
        for c in range(KC):
            nc.vector.tensor_reduce(
                hT_f[:, 32 * c : 32 * (c + 1)],
                xs[:, c, :].rearrange("p (b t) -> p b t", t=T),
                mybir.AxisListType.X, ALU.max,
            )
        hT_b = hTp.tile([128, 128], BF16, tag="hTb")
        nc.vector.tensor_copy(hT_b[:], hT_f[:])

        # h0 in [batch, h] layout via PE transposes
        h0ps = accp.tile([BG, H], BF16, tag="h0ps")
        for c in range(KC):
            nc.tensor.transpose(
                h0ps[:, 128 * c : 128 * (c + 1)],
                hT_f[:, 32 * c : 32 * (c + 1)],
                ident[:],
            )
        h_cur = hp.tile([2 * BG, H], BF16, tag="h")
        nc.vector.tensor_copy(h_cur[0:BG, :], h0ps[:])
        nc.vector.tensor_copy(h_cur[BG : 2 * BG, :], h0ps[:])

        acc = accp.tile([2 * BG, 512], F32)

        pend = None  # (h_new, t) awaiting transposes/acc emission
        for t in range(T_steps):
            rz = rzp.tile([128, 512], F32, tag="rz")
            nn = nnp.tile([128, 512], F32, tag="nn")

            # bias MMs open the accumulation groups (write every element)
            rz_b = nc.tensor.matmul(rz[:], ind4[:], brz[:], start=True, stop=False,
                                    skip_group_check=True)
            nn_b = nc.tensor.matmul(nn[:], ind4[:], bnn[:], start=True, stop=False,
                                    skip_group_check=True)

            # x-projection MMs (independent of the recurrent chain)
            # rz groups: 0=f-r, 1=b-r, 2=f-z, 3=b-z ; nn groups 2,3 = f-sxn, b-sxn
            for c in range(KC):
                for g, (d, j) in enumerate([(0, 0), (1, 0), (0, 1), (1, 1)]):
                    m = 2 * d  # wi of direction d
                    dep(nc.tensor.matmul(
                        rz[32 * g : 32 * (g + 1), :], x_lhsT(d, c, t), w_rhs(m, c, j),
                        start=False, stop=False, skip_group_check=True,
                        tile_position=(0, 32 * g)), rz_b)
                for g, d in ((2, 0), (3, 1)):
                    dep(nc.tensor.matmul(
                        nn[32 * g : 32 * (g + 1), :], x_lhsT(d, c, t), w_rhs(2 * d, c, 2),
                        start=False, stop=(c == KC - 1), skip_group_check=True,
                        tile_position=(0, 32 * g)), nn_b)

            # deferred transposes + masked-mean accumulate for step t-1
            if pend is not None:
                hprev, tprev = pend
                # HW quirk: transpose-mode matmuls crash with base-partition-32
                # inputs, so stage the b-direction rows down to partition 0 first.
                hb0 = ew.tile([BG, H], BF16, tag="hb0")
                nc.vector.tensor_copy(hb0[:], hprev[BG : 2 * BG, :])
                htps = htp.tile([128, 2 * 128], BF16, tag="ht")
                for d in range(2):
                    src = hprev if d == 0 else hb0
                    for c in range(KC):
                        nc.tensor.transpose(
                            htps[:, 128 * d + 32 * c : 128 * d + 32 * (c + 1)],
                            src[0:BG, 128 * c : 128 * (c + 1)],
                            ident[0:BG, 0:BG],
                        )
                mm_a = nc.tensor.matmul(acc[0:BG, :], mdg[0:BG, tprev, :], hprev[0:BG, :],
                                        start=(tprev == 0), stop=False,
                                        skip_group_check=True, tile_position=(0, 0))
                mm_b = nc.tensor.matmul(acc[BG : 2 * BG, :], mdg[BG : 2 * BG, tprev, :],
                                        hprev[BG : 2 * BG, :],
                                        start=(tprev == 0), stop=False,
                                        skip_group_check=True, tile_position=(32, 32))
                if tprev == 0:
                    acc_start = (mm_a, mm_b)
                else:
                    dep(mm_a, acc_start[0])
                    dep(mm_b, acc_start[1])
                hT_f = hTp.tile([128, 128], BF16, tag="hTf")
                nc.vector.tensor_copy(hT_f[:], htps[:, 0:128])
                hT_b = hTp.tile([128, 128], BF16, tag="hTb")
                nc.vector.tensor_copy(hT_b[:], htps[:, 128:256])

            # recurrent MMs
            for c in range(KC):
                for g, (hT, j) in enumerate([(hT_f, 0), (hT_b, 0), (hT_f, 1), (hT_b, 1)]):
                    m = 1 + 2 * (g % 2)  # wh of direction
                    dep(nc.tensor.matmul(
                        rz[32 * g : 32 * (g + 1), :],
                        hT[:, 32 * c : 32 * (c + 1)], w_rhs(m, c, j),
                        start=False, stop=(c == KC - 1), skip_group_check=True,
                        tile_position=(0, 32 * g)), rz_b)
                for g, hT in ((0, hT_f), (1, hT_b)):
                    dep(nc.tensor.matmul(
                        nn[32 * g : 32 * (g + 1), :],
                        hT[:, 32 * c : 32 * (c + 1)], w_rhs(1 + 2 * g, c, 2),
                        start=False, stop=(c == KC - 1), skip_group_check=True,
                        tile_position=(0, 32 * g)), nn_b)

            # ---- elementwise
            rzs = ew.tile([128, 512], BF16, tag="rzs")
            nc.scalar.activation(rzs[:], rz[:], AF.Sigmoid)
            u0 = ew.tile([2 * BG, 512], BF16, tag="u0")
            nc.scalar.activation(u0[:], rz[64:128, :], AF.Sigmoid, scale=-1.0)
            z0 = ew.tile([2 * BG, 512], BF16, tag="z0")
            nc.vector.tensor_copy(z0[:], rzs[64:128, :])
            b2 = ew.tile([2 * BG, 512], BF16, tag="b2")
            nc.vector.tensor_tensor(b2[:], rzs[0:64, :], nn[0:64, :], ALU.mult)
            cc = ew.tile([2 * BG, 512], BF16, tag="cc")
            nc.vector.tensor_tensor(cc[:], b2[:], nn[64:128, :], ALU.add)
            n_t = ew.tile([2 * BG, 512], BF16, tag="nt")
            nc.scalar.activation(n_t[:], cc[:], AF.Tanh)
            w_t = ew.tile([2 * BG, 512], BF16, tag="wt")
            eng_w = nc.gpsimd if use_gpsimd else nc.vector
            eng_w.tensor_tensor(w_t[:], z0[:], h_cur[:], ALU.mult)
            v_t = ew.tile([2 * BG, 512], BF16, tag="vt")
            nc.vector.tensor_tensor(v_t[:], u0[:], n_t[:], ALU.mult)
            h_new = hp.tile([2 * BG, H], BF16, tag="h")
            nc.vector.tensor_tensor(h_new[:], v_t[:], w_t[:], ALU.add)

            pend = (h_new, t)
            h_cur = h_new

        # final masked accumulate for t=63
        hprev, tprev = pend
        dep(nc.tensor.matmul(acc[0:BG, :], mdg[0:BG, tprev, :], hprev[0:BG, :],
                             start=False, stop=True, skip_group_check=True,
                             tile_position=(0, 0)), acc_start[0])
        dep(nc.tensor.matmul(acc[BG : 2 * BG, :], mdg[BG : 2 * BG, tprev, :],
                             hprev[BG : 2 * BG, :], start=False, stop=True,
                             skip_group_check=True, tile_position=(32, 32)),
            acc_start[1])

        outs = cst.tile([2 * BG, H], F32)
        nc.vector.tensor_scalar_mul(outs[:], acc[:], invn[:])
        nc.sync.dma_start(out_d[:], outs[:])


def _prep_inputs(atom_message, w_ih, w_hh, b_ih, b_hh, node_size):
    """Host-side sharding/layout prep. Returns per-core input maps."""
    atom_message = np.asarray(atom_message, dtype=np.float32)
    w_ih = np.asarray(w_ih, dtype=np.float32)
    w_hh = np.asarray(w_hh, dtype=np.float32)
    b_ih = np.asarray(b_ih, dtype=np.float32)
    b_hh = np.asarray(b_hh, dtype=np.float32)
    ns = np.asarray(node_size, dtype=np.int64)

    # ragged -> padded [B, T, H]
    padded = np.zeros((B, T, H), dtype=np.float32)
    starts = np.concatenate([[0], np.cumsum(ns)[:-1]])
    for b in range(B):
        padded[b, : ns[b]] = atom_message[starts[b] : starts[b] + ns[b]]

    # weights (shared across cores)
    w_all = np.stack([w_ih[0].T, w_hh[0].T, w_ih[1].T, w_hh[1].T]).astype(md.bfloat16)
    bias_rz = np.stack([
        b_ih[0, 0:512] + b_hh[0, 0:512],       # f-r
        b_ih[1, 0:512] + b_hh[1, 0:512],       # b-r
        b_ih[0, 512:1024] + b_hh[0, 512:1024], # f-z
        b_ih[1, 512:1024] + b_hh[1, 512:1024], # b-z
    ]).astype(md.bfloat16)
    bias_n = np.stack([
        b_hh[0, 1024:1536], b_hh[1, 1024:1536],  # ghn f, b
        b_ih[0, 1024:1536], b_ih[1, 1024:1536],  # sxn f, b
    ]).astype(md.bfloat16)
    ind4 = np.zeros((4, 128), dtype=np.float32)
    for g in range(4):
        ind4[g, 32 * g : 32 * (g + 1)] = 1.0
    ind4 = ind4.astype(md.bfloat16)
    ident = np.eye(128, dtype=md.bfloat16)

    tgrid = np.arange(T)
    in_maps = []
    for i in range(NC):
        sl = slice(BG * i, BG * (i + 1))
        px = padded[sl]                                   # [32, 64, 512]
        xT = np.ascontiguousarray(px.transpose(2, 0, 1).reshape(H, BG * T))
        nsc = ns[sl]
        mask_f = (tgrid[None, :] < nsc[:, None]).astype(np.float32)       # [32, 64]
        mask_b = (tgrid[None, :] >= (T - nsc)[:, None]).astype(np.float32)
        mdg = np.zeros((2 * BG, T, BG), dtype=np.float32)
        for b in range(BG):
            mdg[b, :, b] = mask_f[b]
            mdg[BG + b, :, b] = mask_b[b]
        inv = np.concatenate([1.0 / nsc, 1.0 / nsc]).astype(np.float32)[:, None]
        in_maps.append({
            "xT": xT.astype(md.bfloat16),
            "w_all": w_all,
            "bias_rz": bias_rz,
            "bias_n": bias_n,
            "ind4": ind4,
            "maskdiag": mdg.astype(md.bfloat16),
            "inv_ns": inv,
            "ident": ident,
        })
    return in_maps


def kernel(atom_message, w_ih, w_hh, b_ih, b_hh, node_size, _sim=False, _trace=False):
    in_maps = _prep_inputs(atom_message, w_ih, w_hh, b_ih, b_hh, node_size)
    import os
    key = (int(os.environ.get("T_STEPS", T)), int(os.environ.get("USE_GPSIMD", 1)), int(os.environ.get("REPEAT", 1)))
    if key not in _PROGRAM_CACHE:
        _PROGRAM_CACHE[key] = build_program(T_steps=key[0], use_gpsimd=bool(key[1]), repeat=key[2])
    nc = _PROGRAM_CACHE[key]

    if _sim:
        from concourse.bass_interp import CoreSim
        sim = CoreSim(nc, require_finite=False, require_nnan=False)
        for k, v in in_maps[0].items():
            sim.tensor(k)[:] = v
        sim.simulate(check_with_hw=False)
        results = [{"out": np.array(sim.tensor("out"))}]
        # simulate only core 0; replicate shape for gather below
        results = results + [None] * (NC - 1)
        res_obj = None
    else:
        res_obj = run_bass_kernel_spmd(nc, in_maps, list(range(NC)), trace=_trace)
        results = res_obj.results

    out = np.zeros((B, 2 * H), dtype=np.float32)
    for i in range(NC):
        r = results[i]
        if r is None:
            continue
        sl = slice(BG * i, BG * (i + 1))
        out[sl, 0:H] = r["out"][0:BG]
        out[sl, H : 2 * H] = r["out"][BG : 2 * BG]
    if _sim:
        return out, None
    return out if not _trace else (out, res_obj)


# revision 10
# speedup vs baseline: 20.0908x; 1.0628x over previous
"""Bidirectional batched GRU over ragged graph sequences — TRN2 Bass/Tile kernel.

Strategy (8 NeuronCores, data-parallel over B=256 graphs -> 32 graphs/core):
  * Both directions' recurrent chains run merged on each core, stacked along
    PSUM/SBUF partitions so the PE's col-group tiling (tile_position) packs
    4 concurrent M=32 matmuls per wave and the elementwise ops process both
    directions in single [64, 512] / [128, 512] instructions.
  * Per step t, PSUM bank `rz` [128,512] holds the 4 groups (f-r, b-r, f-z,
    b-z) = x@wi + h@wh + (bi+bh) accumulated by the PE; bank `nn` holds
    (f-ghn, b-ghn, f-sxn, b-sxn) with ghn = h@wh_n + bh_n, sxn = x@wi_n + bi_n.
  * Gates: sigmoid reads PSUM directly; u = 1-z computed as sigmoid(-s_z).
    n = tanh(sxn + r*ghn);  h' = u*n + z*h.
  * h' is transposed back to [h, batch] layout via 8 tiny PE transposes for
    the next step's stationary operand.
  * Ragged mean-pool: per-step masked accumulation into a persistent PSUM
    bank via diag(mask_t) matmuls; final scale by 1/node_size.
  * h0 = max over padded time = strided tensor_reduce over x.T in SBUF.

All SBUF data is bf16 (PE runs bf16 at 1 cycle/row); PSUM accumulates fp32.
"""
import numpy as np
import ml_dtypes as md
from contextlib import ExitStack

import concourse.bass as bass
import concourse.tile as tile
from concourse import bacc, mybir
from concourse.bass_utils import run_bass_kernel_spmd
from concourse.tile_rust import add_dep_helper

F32 = mybir.dt.float32
BF16 = mybir.dt.bfloat16
AF = mybir.ActivationFunctionType
ALU = mybir.AluOpType

B, H, T = 256, 512, 64
NC = 8
BG = B // NC          # 32 graphs per core
H3 = 3 * H
KC = H // 128         # 4 contraction chunks
NT = H3 // 512        # 3 gate chunks (r, z, n)

_PROGRAM_CACHE = {}


def dep(mm, on):
    add_dep_helper(mm.ins, on.ins, sync=False, reason="psum group order")
    return mm


def build_program(T_steps=T, use_gpsimd=True, repeat=1, loop_n=None):
    nc = bacc.Bacc("TRN2", target_bir_lowering=False, debug=False)

    xT = nc.dram_tensor("xT", [H, BG * T], BF16, kind="ExternalInput").ap()
    w_all = nc.dram_tensor("w_all", [4, H, H3], BF16, kind="ExternalInput").ap()
    bias_rz = nc.dram_tensor("bias_rz", [4, 512], BF16, kind="ExternalInput").ap()
    bias_n = nc.dram_tensor("bias_n", [4, 512], BF16, kind="ExternalInput").ap()
    ind4_d = nc.dram_tensor("ind4", [4, 128], BF16, kind="ExternalInput").ap()
    maskdiag = nc.dram_tensor("maskdiag", [2 * BG, T, BG], BF16, kind="ExternalInput").ap()
    inv_ns = nc.dram_tensor("inv_ns", [2 * BG, 1], F32, kind="ExternalInput").ap()
    ident_d = nc.dram_tensor("ident", [128, 128], BF16, kind="ExternalInput").ap()
    out_d = nc.dram_tensor("out", [2 * BG, H], F32, kind="ExternalOutput").ap()

    with tile.TileContext(nc) as tc, ExitStack() as ctx:
        cst = ctx.enter_context(tc.tile_pool(name="cst", bufs=1))
        hTp = ctx.enter_context(tc.tile_pool(name="hTp", bufs=3))
        hp = ctx.enter_context(tc.tile_pool(name="hp", bufs=3))
        ew = ctx.enter_context(tc.tile_pool(name="ew", bufs=2))
        rzp = ctx.enter_context(tc.tile_pool(name="rzp", bufs=2, space="PSUM"))
        nnp = ctx.enter_context(tc.tile_pool(name="nnp", bufs=2, space="PSUM"))
        htp = ctx.enter_context(tc.tile_pool(name="htp", bufs=2, space="PSUM"))
        accp = ctx.enter_context(tc.tile_pool(name="accp", bufs=1, space="PSUM"))

        # ---- constants / inputs to SBUF
        xs = cst.tile([128, KC, BG * T], BF16)           # x.T  [h%128, h//128, (b,t)]
        nc.sync.dma_start(xs[:], xT.rearrange("(c p) r -> p c r", p=128))
        ws = cst.tile([128, 4, KC, H3], BF16)            # wiT_f, whT_f, wiT_b, whT_b
        nc.sync.dma_start(ws[:], w_all.rearrange("m (c p) n -> p m c n", p=128))
        brz = cst.tile([4, 512], BF16)
        nc.sync.dma_start(brz[:], bias_rz[:])
        bnn = cst.tile([4, 512], BF16)
        nc.sync.dma_start(bnn[:], bias_n[:])
        ind4 = cst.tile([4, 128], BF16)
        nc.sync.dma_start(ind4[:], ind4_d[:])
        mdg = cst.tile([2 * BG, T, BG], BF16)
        nc.sync.dma_start(mdg[:], maskdiag[:])
        invn = cst.tile([2 * BG, 1], F32)
        nc.sync.dma_start(invn[:], inv_ns[:])
        ident = cst.tile([128, 128], BF16)
        nc.sync.dma_start(ident[:], ident_d[:])

        def x_lhsT(d, c, t):
            # stationary operand [128, BG] for direction d, k-chunk c, step t
            tt = t if d == 0 else T - 1 - t
            return xs[:, c, :].rearrange("p (b t) -> p t b", t=T)[:, tt, :]

        def w_rhs(m, c, j):
            return ws[:, m, c, 512 * j : 512 * (j + 1)]

        if loop_n is not None:
            with tc.For_i(0, loop_n, 1):
                body(nc, tc, ctx, T_steps, use_gpsimd, xs, ws, brz, bnn, ind4,
                     mdg, invn, ident, out_d, x_lhsT, w_rhs, cst, hTp, hp, ew,
                     rzp, nnp, htp, accp)
        else:
            for _rep in range(repeat):
                body(nc, tc, ctx, T_steps, use_gpsimd, xs, ws, brz, bnn, ind4,
                     mdg, invn, ident, out_d, x_lhsT, w_rhs, cst, hTp, hp, ew,
                     rzp, nnp, htp, accp)

    nc.compile()
    return nc


def body(nc, tc, ctx, T_steps, use_gpsimd, xs, ws, brz, bnn, ind4, mdg, invn,
         ident, out_d, x_lhsT, w_rhs, cst, hTp, hp, ew, rzp, nnp, htp, accp):
    if True:
        # ---- h0 = max over padded time, directly in transposed layout
        hT_f = hTp.tile([128, 128], BF16, tag="hTf")
        for c in range(KC):
            nc.vector.tensor_reduce(
                hT_f[:, 32 * c : 32 * (c + 1)],
                xs[:, c, :].rearrange("p (b t) -> p b t", t=T),
                mybir.AxisListType.X, ALU.max,
            )
        hT_b = hTp.tile([128, 128], BF16, tag="hTb")
        nc.vector.tensor_copy(hT_b[:], hT_f[:])

        # h0 in [batch, h] layout via PE transposes
        h0ps = accp.tile([BG, H], BF16, tag="h0ps")
        for c in range(KC):
            nc.tensor.transpose(
                h0ps[:, 128 * c : 128 * (c + 1)],
                hT_f[:, 32 * c : 32 * (c + 1)],
                ident[:],
            )
        h_cur = hp.tile([2 * BG, H], BF16, tag="h")
        nc.vector.tensor_copy(h_cur[0:BG, :], h0ps[:])
        nc.vector.tensor_copy(h_cur[BG : 2 * BG, :], h0ps[:])

        acc = accp.tile([2 * BG, 512], F32)

        pend = None  # (h_new, t) awaiting transposes/acc emission
        for t in range(T_steps):
            rz = rzp.tile([128, 512], F32, tag="rz")
            nn = nnp.tile([128, 512], F32, tag="nn")

            # bias MMs open the accumulation groups (write every element)
            rz_b = nc.tensor.matmul(rz[:], ind4[:], brz[:], start=True, stop=False,
                                    skip_group_check=True)
            nn_b = nc.tensor.matmul(nn[:], ind4[:], bnn[:], start=True, stop=False,
                                    skip_group_check=True)

            # x-projection MMs (independent of the recurrent chain)
            # rz groups: 0=f-r, 1=b-r, 2=f-z, 3=b-z ; nn groups 2,3 = f-sxn, b-sxn
            # same-lhsT MMs adjacent: r, z, n chunks share the stationary x tile
            for c in range(KC):
                for d in range(2):
                    lx = x_lhsT(d, c, t)
                    for g, j in ((0 + d, 0), (2 + d, 1)):
                        dep(nc.tensor.matmul(
                            rz[32 * g : 32 * (g + 1), :], lx, w_rhs(2 * d, c, j),
                            start=False, stop=False, skip_group_check=True,
                            tile_position=(0, 32 * g)), rz_b)
                    g = 2 + d
                    dep(nc.tensor.matmul(
                        nn[32 * g : 32 * (g + 1), :], lx, w_rhs(2 * d, c, 2),
                        start=False, stop=(c == KC - 1), skip_group_check=True,
                        tile_position=(0, 32 * g)), nn_b)

            # deferred transposes + masked-mean accumulate for step t-1
            if pend is not None:
                hprev, tprev = pend
                # HW quirk: transpose-mode matmuls crash with base-partition-32
                # inputs, so stage the b-direction rows down to partition 0 first.
                hb0 = ew.tile([BG, H], BF16, tag="hb0")
                nc.vector.tensor_copy(hb0[:], hprev[BG : 2 * BG, :])
                htps = htp.tile([128, 2 * 128], BF16, tag="ht")
                for d in range(2):
                    src = hprev if d == 0 else hb0
                    for c in range(KC):
                        nc.tensor.transpose(
                            htps[:, 128 * d + 32 * c : 128 * d + 32 * (c + 1)],
                            src[0:BG, 128 * c : 128 * (c + 1)],
                            ident[0:BG, 0:BG],
                        )
                mm_a = nc.tensor.matmul(acc[0:BG, :], mdg[0:BG, tprev, :], hprev[0:BG, :],
                                        start=(tprev == 0), stop=False,
                                        skip_group_check=True, tile_position=(0, 0))
                mm_b = nc.tensor.matmul(acc[BG : 2 * BG, :], mdg[BG : 2 * BG, tprev, :],
                                        hprev[BG : 2 * BG, :],
                                        start=(tprev == 0), stop=False,
                                        skip_group_check=True, tile_position=(32, 32))
                if tprev == 0:
                    acc_start = (mm_a, mm_b)
                else:
                    dep(mm_a, acc_start[0])
                    dep(mm_b, acc_start[1])
                hT_f = hTp.tile([128, 128], BF16, tag="hTf")
                nc.vector.tensor_copy(hT_f[:], htps[:, 0:128])
                hT_b = hTp.tile([128, 128], BF16, tag="hTb")
                nc.vector.tensor_copy(hT_b[:], htps[:, 128:256])

            # recurrent MMs
            for c in range(KC):
                for d, hT in ((0, hT_f), (1, hT_b)):
                    lh = hT[:, 32 * c : 32 * (c + 1)]
                    for g, j in ((0 + d, 0), (2 + d, 1)):
                        dep(nc.tensor.matmul(
                            rz[32 * g : 32 * (g + 1), :], lh, w_rhs(1 + 2 * d, c, j),
                            start=False, stop=(c == KC - 1), skip_group_check=True,
                            tile_position=(0, 32 * g)), rz_b)
                    dep(nc.tensor.matmul(
                        nn[32 * d : 32 * (d + 1), :], lh, w_rhs(1 + 2 * d, c, 2),
                        start=False, stop=(c == KC - 1), skip_group_check=True,
                        tile_position=(0, 32 * d)), nn_b)

            # ---- elementwise
            rzs = ew.tile([128, 512], BF16, tag="rzs")
            nc.scalar.activation(rzs[:], rz[:], AF.Sigmoid)
            u0 = ew.tile([2 * BG, 512], BF16, tag="u0")
            nc.scalar.activation(u0[:], rz[64:128, :], AF.Sigmoid, scale=-1.0)
            z0 = ew.tile([2 * BG, 512], BF16, tag="z0")
            nc.vector.tensor_copy(z0[:], rzs[64:128, :])
            b2 = ew.tile([2 * BG, 512], BF16, tag="b2")
            nc.vector.tensor_tensor(b2[:], rzs[0:64, :], nn[0:64, :], ALU.mult)
            cc = ew.tile([2 * BG, 512], BF16, tag="cc")
            nc.vector.tensor_tensor(cc[:], b2[:], nn[64:128, :], ALU.add)
            n_t = ew.tile([2 * BG, 512], BF16, tag="nt")
            nc.scalar.activation(n_t[:], cc[:], AF.Tanh)
            w_t = ew.tile([2 * BG, 512], BF16, tag="wt")
            eng_w = nc.gpsimd if use_gpsimd else nc.vector
            eng_w.tensor_tensor(w_t[:], z0[:], h_cur[:], ALU.mult)
            v_t = ew.tile([2 * BG, 512], BF16, tag="vt")
            nc.vector.tensor_tensor(v_t[:], u0[:], n_t[:], ALU.mult)
            h_new = hp.tile([2 * BG, H], BF16, tag="h")
            nc.vector.tensor_tensor(h_new[:], v_t[:], w_t[:], ALU.add)

            pend = (h_new, t)
            h_cur = h_new

        # final masked accumulate for t=63
        hprev, tprev = pend
        dep(nc.tensor.matmul(acc[0:BG, :], mdg[0:BG, tprev, :], hprev[0:BG, :],
                             start=False, stop=True, skip_group_check=True,
                             tile_position=(0, 0)), acc_start[0])
        dep(nc.tensor.matmul(acc[BG : 2 * BG, :], mdg[BG : 2 * BG, tprev, :],
                             hprev[BG : 2 * BG, :], start=False, stop=True,
                             skip_group_check=True, tile_position=(32, 32)),
            acc_start[1])

        outs = cst.tile([2 * BG, H], F32)
        nc.vector.tensor_scalar_mul(outs[:], acc[:], invn[:])
        nc.sync.dma_start(out_d[:], outs[:])


def _prep_inputs(atom_message, w_ih, w_hh, b_ih, b_hh, node_size):
    """Host-side sharding/layout prep. Returns per-core input maps."""
    atom_message = np.asarray(atom_message, dtype=np.float32)
    w_ih = np.asarray(w_ih, dtype=np.float32)
    w_hh = np.asarray(w_hh, dtype=np.float32)
    b_ih = np.asarray(b_ih, dtype=np.float32)
    b_hh = np.asarray(b_hh, dtype=np.float32)
    ns = np.asarray(node_size, dtype=np.int64)

    # ragged -> padded [B, T, H]
    padded = np.zeros((B, T, H), dtype=np.float32)
    starts = np.concatenate([[0], np.cumsum(ns)[:-1]])
    for b in range(B):
        padded[b, : ns[b]] = atom_message[starts[b] : starts[b] + ns[b]]

    # weights (shared across cores)
    w_all = np.stack([w_ih[0].T, w_hh[0].T, w_ih[1].T, w_hh[1].T]).astype(md.bfloat16)
    bias_rz = np.stack([
        b_ih[0, 0:512] + b_hh[0, 0:512],       # f-r
        b_ih[1, 0:512] + b_hh[1, 0:512],       # b-r
        b_ih[0, 512:1024] + b_hh[0, 512:1024], # f-z
        b_ih[1, 512:1024] + b_hh[1, 512:1024], # b-z
    ]).astype(md.bfloat16)
    bias_n = np.stack([
        b_hh[0, 1024:1536], b_hh[1, 1024:1536],  # ghn f, b
        b_ih[0, 1024:1536], b_ih[1, 1024:1536],  # sxn f, b
    ]).astype(md.bfloat16)
    ind4 = np.zeros((4, 128), dtype=np.float32)
    for g in range(4):
        ind4[g, 32 * g : 32 * (g + 1)] = 1.0
    ind4 = ind4.astype(md.bfloat16)
    ident = np.eye(128, dtype=md.bfloat16)

    tgrid = np.arange(T)
    in_maps = []
    for i in range(NC):
        sl = slice(BG * i, BG * (i + 1))
        px = padded[sl]                                   # [32, 64, 512]
        xT = np.ascontiguousarray(px.transpose(2, 0, 1).reshape(H, BG * T))
        nsc = ns[sl]
        mask_f = (tgrid[None, :] < nsc[:, None]).astype(np.float32)       # [32, 64]
        mask_b = (tgrid[None, :] >= (T - nsc)[:, None]).astype(np.float32)
        mdg = np.zeros((2 * BG, T, BG), dtype=np.float32)
        for b in range(BG):
            mdg[b, :, b] = mask_f[b]
            mdg[BG + b, :, b] = mask_b[b]
        inv = np.concatenate([1.0 / nsc, 1.0 / nsc]).astype(np.float32)[:, None]
        in_maps.append({
            "xT": xT.astype(md.bfloat16),
            "w_all": w_all,
            "bias_rz": bias_rz,
            "bias_n": bias_n,
            "ind4": ind4,
            "maskdiag": mdg.astype(md.bfloat16),
            "inv_ns": inv,
            "ident": ident,
        })
    return in_maps


def kernel(atom_message, w_ih, w_hh, b_ih, b_hh, node_size, _sim=False, _trace=False):
    in_maps = _prep_inputs(atom_message, w_ih, w_hh, b_ih, b_hh, node_size)
    import os
    key = (int(os.environ.get("T_STEPS", T)), int(os.environ.get("USE_GPSIMD", 1)),
           int(os.environ.get("REPEAT", 1))) if os.environ.get("KBENCH") else (T, 1, 1)
    if key not in _PROGRAM_CACHE:
        _PROGRAM_CACHE[key] = build_program(T_steps=key[0], use_gpsimd=bool(key[1]), repeat=key[2])
    nc = _PROGRAM_CACHE[key]

    if _sim:
        from concourse.bass_interp import CoreSim
        sim = CoreSim(nc, require_finite=False, require_nnan=False)
        for k, v in in_maps[0].items():
            sim.tensor(k)[:] = v
        sim.simulate(check_with_hw=False)
        results = [{"out": np.array(sim.tensor("out"))}]
        # simulate only core 0; replicate shape for gather below
        results = results + [None] * (NC - 1)
        res_obj = None
    else:
        res_obj = run_bass_kernel_spmd(nc, in_maps, list(range(NC)), trace=_trace)
        results = res_obj.results

    out = np.zeros((B, 2 * H), dtype=np.float32)
    for i in range(NC):
        r = results[i]
        if r is None:
            continue
        sl = slice(BG * i, BG * (i + 1))
        out[sl, 0:H] = r["out"][0:BG]
        out[sl, H : 2 * H] = r["out"][BG : 2 * BG]
    if _sim:
        return out, None
    return out if not _trace else (out, res_obj)
